# revision 1
# baseline (speedup 1.0000x reference)
"""YOLO-style DetectionLoss on 8 Trainium2 NeuronCores (Bass/Tile).

Pure data parallelism: batch 8192 -> 1024 per core. Per core the
1024*7*7 = 50176 cells are laid out as 128 SBUF partitions x 392 cells
(each partition owns a contiguous run of 8 batch images = 392 cells of
35 fp32 channels). All per-cell math is elementwise along the free dim
with strided/broadcast access patterns spread across DVE/ACT/GPSIMD;
the four loss groups are reduced on the fly by ACT square+accumulate
ops into per-partition accumulators, which are DMA'd out
([128, 4*NCHUNKS] per core) and summed on the host (the "all-reduce"
of the scalar loss).
"""

import os

os.environ.setdefault("JAX_COMPILATION_CACHE_DIR", "/tmp/jaxcache")
os.environ.setdefault("JAX_PERSISTENT_CACHE_MIN_COMPILE_TIME_SECS", "1")
os.environ.setdefault("JAX_PERSISTENT_CACHE_MIN_ENTRY_SIZE_BYTES", "0")

import numpy as np

import concourse.bacc as bacc
import concourse.mybir as mybir
import concourse.tile as tile
from concourse.bass_utils import run_bass_kernel_spmd

F32 = mybir.dt.float32
AF = mybir.ActivationFunctionType
OP = mybir.AluOpType

NB, C, S = 3, 20, 7
D = 5 * NB + C                 # 35
B = 8192
NCORES = 8
P = 128

COORD_SCALE, NOOBJ_SCALE = 5.0, 0.5
NTERMS = 4                     # xywh, contain, noobj, class


def default_chunks(kpp):
    """Chunk sizes (cells per partition per chunk). A small first chunk
    shortens the pipeline-fill stall; a smaller last chunk shortens the
    tail. Chunk size is capped at 98 to bound SBUF temp usage."""
    if kpp == 392:
        return [28, 98, 98, 98, 70]
    if kpp % 98 == 0:
        return [98] * (kpp // 98)
    if kpp % 49 == 0:
        return [49] * (kpp // 49)
    if kpp % 7 == 0:
        return [7] * (kpp // 7)
    return [kpp]


def build_nc(bc: int, ks=None, repeats: int = 1, io_bufs: int = 2,
             loop_repeats: int = 0, parts: str = "full"):
    """Trace the per-core Bass program for a per-core batch of `bc`.

    `repeats` re-runs the whole computation (same data, same result) to
    support slope-based device timing; output is unchanged.
    """
    cells = bc * S * S
    assert cells % P == 0
    kpp = cells // P               # cells per partition
    if ks is None:
        ks = default_chunks(kpp)
    assert sum(ks) == kpp
    nchunks = len(ks)
    kmax = max(ks)

    nc = bacc.Bacc("TRN2", debug=False, num_devices=NCORES)
    out_h = nc.dram_tensor("output", [bc, S, S, D], F32, kind="ExternalInput")
    tgt_h = nc.dram_tensor("target", [bc, S, S, D], F32, kind="ExternalInput")
    acc_h = nc.dram_tensor("acc", [P, NTERMS * nchunks], F32,
                           kind="ExternalOutput")

    # partition p owns bc/P consecutive batch images -> contiguous DMA rows
    out_v = out_h.ap().rearrange("(p a) h w d -> p (a h w d)", p=P)
    tgt_v = tgt_h.ap().rearrange("(p a) h w d -> p (a h w d)", p=P)

    with tile.TileContext(nc) as tc:
        with (
            tc.tile_pool(name="io", bufs=io_bufs) as io_pool,
            tc.tile_pool(name="p6", bufs=9) as p6,
            tc.tile_pool(name="p12", bufs=2) as p12,
            tc.tile_pool(name="p3", bufs=8) as p3,
            tc.tile_pool(name="psc", bufs=2) as psc,
            tc.tile_pool(name="p1", bufs=8) as p1,
            tc.tile_pool(name="p20", bufs=2) as p20,
            tc.tile_pool(name="psq", bufs=2) as psq,
            tc.tile_pool(name="accp", bufs=1) as accp,
        ):
            acc = accp.tile([P, NTERMS * nchunks], F32)

            import contextlib
            loop_cm = (tc.For_i(0, loop_repeats, 1) if loop_repeats
                       else contextlib.nullcontext())
            with loop_cm:
                for rep in range(repeats):
                    off = 0
                    for ci, k in enumerate(ks):
                        ot = io_pool.tile([P, k * D], F32, name="ot", tag="ot")
                        tt = io_pool.tile([P, k * D], F32, name="tt", tag="tt")
                        nc.sync.dma_start(ot[:], out_v[:, off:off + k * D])
                        nc.sync.dma_start(tt[:], tgt_v[:, off:off + k * D])
                        off += k * D

                        o3 = ot[:].rearrange("p (k d) -> p k d", d=D)
                        t3 = tt[:].rearrange("p (k d) -> p k d", d=D)
                        ob = o3[:, :, 0:15].rearrange("p k (b f) -> p k b f", f=5)
                        tb = t3[:, :, 0:15].rearrange("p k (b f) -> p k b f", f=5)

                        pxy = ob[:, :, :, 0:2]          # [P,k,3,2]
                        pwh = ob[:, :, :, 2:4]
                        pc_ = ob[:, :, :, 4]            # [P,k,3]
                        pcls = o3[:, :, 15:35]          # [P,k,20]
                        txy = tb[:, :, :, 0:2]
                        twh = tb[:, :, :, 2:4]
                        tcls = t3[:, :, 15:35]
                        t0 = tb[:, :, 0, :]             # [P,k,5] target box 0
                        tw0 = t0[:, :, 2]
                        th0 = t0[:, :, 3]
                        conf = t0[:, :, 4]              # [P,k] exactly 0/1

                        txy0b = t0[:, :, 0:2].unsqueeze(2).broadcast_to(
                            [P, k, 3, 2])
                        twh0b = t0[:, :, 2:4].unsqueeze(2).broadcast_to(
                            [P, k, 3, 2])
                        conf3 = conf.unsqueeze(2).broadcast_to([P, k, 3])
                        conf20 = conf.unsqueeze(2).broadcast_to([P, k, 20])

                        def t6(name):
                            return p6.tile([P, k, 3, 2], F32, name=name,
                                           tag="t6")[:]

                        def t3k(name):
                            return p3.tile([P, k, 3], F32, name=name, tag="t3")[:]

                        def t1k(name):
                            return p1.tile([P, k], F32, name=name, tag="t1")[:]

                        def slot(term):
                            i = ci * NTERMS + term
                            return acc[:, i:i + 1]

                        # Engines run in order, so issue ops in pipeline-stage
                        # order per engine: early ACT (sqrt/abs/nob) before the
                        # DVE chain needs them, Pool ops that only need DMA
                        # next, chunk-closing ACT squares last.
                        do_class = parts in ("full", "classonly")
                        do_main = parts in ("full", "noclass")
                        if parts == "dmaonly":
                            sqk = psc.tile([P, k, 3], F32, name="sqk",
                                           tag="sqc")[:]
                            nc.scalar.activation(
                                sqk, ob[:, :, :, 4], AF.Square,
                                accum_out=slot(0))
                            sqk2 = psc.tile([P, k, 3], F32, name="sqk2",
                                            tag="sqc")[:]
                            nc.scalar.activation(
                                sqk2, tb[:, :, :, 4], AF.Square,
                                accum_out=slot(1))

                        # DVE: center diff feeds ACT abs early
                        if do_main:
                            dcx = t6("dcx")
                            nc.vector.tensor_sub(dcx, pxy, txy0b)

                            # early ACT batch (DMA/dcx deps only)
                            sp = t6("sp")
                            nc.scalar.activation(sp, pwh, AF.Sqrt)
                            st = t6("st")
                            nc.scalar.activation(st, twh, AF.Sqrt)
                            nc.scalar.activation(
                                dcx, dcx, AF.Abs, scale=2.0 / S)


                        # Pool: early DMA-only products for the DVE chain,
                        # then class diff+mask, then wh sqrt diff
                        if do_main:
                            a1 = t3k("a1")
                            nc.gpsimd.tensor_mul(
                                a1, ob[:, :, :, 2], ob[:, :, :, 3])
                            a2 = t1k("a2")
                            nc.gpsimd.tensor_mul(a2, tw0, th0)
                            dxy = t6("dxy")
                            nc.gpsimd.tensor_sub(dxy, pxy, txy)
                        if do_class:
                            dcl = p20.tile([P, k, 20], F32, name="dcl",
                                           tag="dcl")[:]
                            nc.gpsimd.tensor_sub(dcl, pcls, tcls)
                            dclm = p20.tile([P, k, 20], F32, name="dclm",
                                            tag="dclm")[:]
                            nc.gpsimd.tensor_mul(dclm, dcl, conf20)
                        if do_main:
                            dwh = t6("dwh")
                            nc.gpsimd.tensor_sub(dwh, sp, st)

                        if do_main:
                            # ---- IoU(pred box b, target box 0), b=0..2 ----
                            # per-axis overlap (x2 units):
                            #   ov2 = min(2*min(pw,tw), pw+tw - |2*(px-tx)/S|)
                            spt = t6("spt")
                            nc.vector.tensor_add(spt, pwh, twh0b)
                            nc.vector.tensor_sub(spt, spt, dcx)       # u, in place
                            mn = t6("mn")
                            nc.vector.tensor_tensor(mn, pwh, twh0b, op=OP.min)
                            nc.vector.scalar_tensor_tensor(           # ov, in place
                                mn, mn, 2.0, spt, op0=OP.mult, op1=OP.min)
                            nc.scalar.activation(mn, mn, AF.Relu)     # dpos, in place
                            inter = t3k("inter")                      # inter4 = 4*I
                            nc.vector.tensor_mul(inter, mn[:, :, :, 0], mn[:, :, :, 1])
                            nc.vector.tensor_add(                     # s, in place
                                a1, a1, a2.unsqueeze(2).broadcast_to([P, k, 3]))
                            nc.vector.scalar_tensor_tensor(           # den4, in place
                                a1, a1, 4.0, inter, op0=OP.mult, op1=OP.subtract)
                            rcp = t3k("rcp")
                            nc.vector.reciprocal(rcp, a1)
                            iou = inter                               # in place
                            nc.vector.tensor_mul(iou, inter, rcp)

                            # ---- responsible-box one-hot (first argmax) ----
                            i0, i1, i2 = iou[:, :, 0], iou[:, :, 1], iou[:, :, 2]
                            c01 = t1k("c01")
                            nc.vector.tensor_tensor(c01, i0, i1, op=OP.is_ge)
                            c02 = t1k("c02")
                            nc.vector.tensor_tensor(c02, i0, i2, op=OP.is_ge)
                            c12 = t1k("c12")
                            nc.vector.tensor_tensor(c12, i1, i2, op=OP.is_ge)
                            rm = t3k("rm")
                            r0, r1, r2 = rm[:, :, 0], rm[:, :, 1], rm[:, :, 2]
                            nc.vector.tensor_mul(r0, c01, c02)
                            # r1 = (1-e0)*c12  via (e0 != 1) * c12
                            nc.vector.scalar_tensor_tensor(
                                r1, r0, 1.0, c12, op0=OP.not_equal, op1=OP.mult)
                            t01 = t1k("t01")
                            nc.vector.tensor_add(t01, r0, r1)
                            nc.vector.tensor_scalar(
                                r2, t01, 1.0, None, op0=OP.not_equal)
                            nc.vector.tensor_mul(rm, rm, conf3)       # obj mask
                            pcm = t3k("pcm")
                            nc.vector.scalar_tensor_tensor(
                                pcm, conf3, 1.0, pc_,
                                op0=OP.not_equal, op1=OP.mult)
                            rm2 = rm.unsqueeze(3).broadcast_to([P, k, 3, 2])

                            # ---- masked residuals on DVE ----
                            cw = p12.tile([P, k, 3, 4], F32, name="cw", tag="t12")[:]
                            nc.vector.tensor_mul(cw[:, :, :, 0:2], dxy, rm2)
                            nc.vector.tensor_mul(cw[:, :, :, 2:4], dwh, rm2)
                            nc.vector.tensor_sub(iou, pc_, iou)       # dc, in place
                            nc.vector.tensor_mul(iou, iou, rm)        # dcm, in place

                            # ---- chunk-closing ACT square+accumulate ----
                            sqb = p12.tile([P, k, 3, 4], F32, name="sqb", tag="sqb")[:]
                            nc.scalar.activation(sqb, cw, AF.Square,
                                                 accum_out=slot(0))
                            sqd = psc.tile([P, k, 3], F32, name="sqd", tag="sqc")[:]
                            nc.scalar.activation(sqd, iou, AF.Square,
                                                 accum_out=slot(1))
                            sqc = psc.tile([P, k, 3], F32, name="sqc", tag="sqc")[:]
                            nc.scalar.activation(sqc, pcm, AF.Square,
                                                 accum_out=slot(2))

                        if do_class:
                            sqa = psq.tile([P, k, 20], F32, name="sqa",
                                           tag="sqa")[:]
                            nc.scalar.activation(sqa, dclm, AF.Square,
                                                 accum_out=slot(3))


            nc.sync.dma_start(acc_h.ap()[:], acc[:])

    nc.compile()
    return nc


_CACHE = {}


def _get_nc(bc, ks=None, repeats=1, io_bufs=2, loop_repeats=0, parts="full"):
    key = (bc, tuple(ks) if ks else None, repeats, io_bufs, loop_repeats, parts)
    if key not in _CACHE:
        _CACHE[key] = build_nc(bc, ks, repeats, io_bufs, loop_repeats, parts)
    return _CACHE[key]


def combine_acc(acc_list, nchunks):
    """Host-side gather: fold per-(core,partition,chunk) term sums into the
    scalar loss exactly as the reference's final weighted sum does."""
    tot = np.zeros(NTERMS, dtype=np.float64)
    for a in acc_list:
        tot += a.astype(np.float64).reshape(P, nchunks, NTERMS).sum(axis=(0, 1))
    xywh, cont, noobj, cls = tot
    loss = (COORD_SCALE * xywh + cont + NOOBJ_SCALE * noobj + cls) / B
    return np.float32(loss)


BEST_KS = [49] * 8
BEST_IO_BUFS = 3


def kernel(output: np.ndarray, target: np.ndarray) -> np.ndarray:
    assert output.shape == (B, S, S, D) and target.shape == (B, S, S, D)
    bc = B // NCORES
    nchunks = len(BEST_KS)
    nc = _get_nc(bc, BEST_KS, io_bufs=BEST_IO_BUFS)
    in_maps = [
        {
            "output": np.ascontiguousarray(output[i * bc:(i + 1) * bc]),
            "target": np.ascontiguousarray(target[i * bc:(i + 1) * bc]),
        }
        for i in range(NCORES)
    ]
    res = run_bass_kernel_spmd(nc, in_maps, list(range(NCORES)))
    return combine_acc([r["acc"] for r in res.results], nchunks)



# revision 14
# speedup vs baseline: 1.0816x; 1.0816x over previous
"""YOLO-style DetectionLoss on 8 Trainium2 NeuronCores (Bass/Tile).

Pure data parallelism: batch 8192 -> 1024 per core; 1024*7*7 = 50176
cells laid out as 128 SBUF partitions x 392 cells (each partition owns 8
consecutive images). Per chunk of k cells the kernel builds masked
residual tiles whose squares sum to the loss:

  Vbox[...,b,0:4] = sqrt(5)*resp_b*(dxy | dwh)   (xy + wh terms)
  Vbox[...,b,4]   = resp_b*(pc_b - max_iou)      (contain term)
  Vcls[...,c]     = obj*(pcls_c - tcls_c)        (class term)
  noobj term      = 0.5*noobj*pc_b^2 via Square(pc)+masked accumulate

All loss weights are folded into the masks, so each chunk closes with
two ACT Square+accumulate ops (Vbox, Vcls) plus one Pool masked
accumulate -> 3 accumulator slots per chunk, summed on the host and
divided by the global batch.

The responsible-box one-hot uses reduce_max + is_equal (exact fp match)
instead of pairwise compares; ties can only occur when every IoU in a
cell is exactly 0 (measure-zero effect on the loss).
"""

import os

os.environ.setdefault("JAX_COMPILATION_CACHE_DIR", "/tmp/jaxcache")
os.environ.setdefault("JAX_PERSISTENT_CACHE_MIN_COMPILE_TIME_SECS", "1")
os.environ.setdefault("JAX_PERSISTENT_CACHE_MIN_ENTRY_SIZE_BYTES", "0")

import numpy as np

import concourse.bacc as bacc
import concourse.mybir as mybir
import concourse.tile as tile
from concourse.bass_utils import run_bass_kernel_spmd

F32 = mybir.dt.float32
AF = mybir.ActivationFunctionType
OP = mybir.AluOpType
AX = mybir.AxisListType

NB, C, S = 3, 20, 7
D = 5 * NB + C                 # 35
B = 8192
NCORES = 8
P = 128

SQRT5 = 5.0 ** 0.5
NTERMS = 3                     # box(xy+wh+contain), class, noobj


def default_chunks(kpp):
    if kpp == 392:
        return [98, 147, 147]
    if kpp % 98 == 0:
        return [98] * (kpp // 98)
    if kpp % 49 == 0:
        return [49] * (kpp // 49)
    return [kpp]


def build_nc(bc: int, ks=None, repeats: int = 1, io_bufs: int = 2,
             loop_repeats: int = 0, use_reduce: bool = True,
             use_ttr: bool = False, use_ts2: bool = True):
    """Trace the per-core Bass program for a per-core batch of `bc`."""
    cells = bc * S * S
    assert cells % P == 0
    kpp = cells // P
    if ks is None:
        ks = default_chunks(kpp)
    assert sum(ks) == kpp
    nchunks = len(ks)

    nc = bacc.Bacc("TRN2", debug=False, num_devices=NCORES)
    out_h = nc.dram_tensor("output", [bc, S, S, D], F32, kind="ExternalInput")
    tgt_h = nc.dram_tensor("target", [bc, S, S, D], F32, kind="ExternalInput")
    acc_h = nc.dram_tensor("acc", [P, NTERMS * nchunks], F32,
                           kind="ExternalOutput")

    out_v = out_h.ap().rearrange("(p a) h w d -> p (a h w d)", p=P)
    tgt_v = tgt_h.ap().rearrange("(p a) h w d -> p (a h w d)", p=P)

    with tile.TileContext(nc) as tc:
        with (
            tc.tile_pool(name="io", bufs=io_bufs) as io_pool,
            tc.tile_pool(name="pv", bufs=2) as pv,       # Vbox
            tc.tile_pool(name="pvc", bufs=2) as pvc,     # Vcls
            tc.tile_pool(name="p6", bufs=2) as p6,       # [k,3,2] temps
            tc.tile_pool(name="pw", bufs=2) as pw_pool,  # dwt [k,3,4]
            tc.tile_pool(name="psqrt", bufs=2) as psqrt, # sp/st
            tc.tile_pool(name="p3", bufs=2) as p3,       # [k,3] temps
            tc.tile_pool(name="p1", bufs=2) as p1,       # [k] temps
            tc.tile_pool(name="accp", bufs=1) as accp,
        ):
            acc = accp.tile([P, NTERMS * nchunks], F32)

            import contextlib
            loop_cm = (tc.For_i(0, loop_repeats, 1) if loop_repeats
                       else contextlib.nullcontext())
            with loop_cm:
                for rep in range(repeats):
                    off = 0
                    for ci, k in enumerate(ks):
                        ot = io_pool.tile([P, k * D], F32, name="ot", tag="ot")
                        tt = io_pool.tile([P, k * D], F32, name="tt", tag="tt")
                        nc.sync.dma_start(ot[:], out_v[:, off:off + k * D])
                        nc.sync.dma_start(tt[:], tgt_v[:, off:off + k * D])
                        off += k * D

                        o3 = ot[:].rearrange("p (k d) -> p k d", d=D)
                        t3 = tt[:].rearrange("p (k d) -> p k d", d=D)
                        ob = o3[:, :, 0:15].rearrange("p k (b f) -> p k b f", f=5)
                        tb = t3[:, :, 0:15].rearrange("p k (b f) -> p k b f", f=5)

                        pxy = ob[:, :, :, 0:2]
                        pwh = ob[:, :, :, 2:4]
                        pc_ = ob[:, :, :, 4]
                        twh = tb[:, :, :, 2:4]
                        t0 = tb[:, :, 0, :]
                        tw0 = t3[:, :, 2]
                        th0 = t3[:, :, 3]
                        conf = t3[:, :, 4]
                        ocls = o3[:, :, 15:35]
                        tcls = t3[:, :, 15:35]

                        txy0b = t0[:, :, 0:2].unsqueeze(2).broadcast_to(
                            [P, k, 3, 2])
                        twh0b = t0[:, :, 2:4].unsqueeze(2).broadcast_to(
                            [P, k, 3, 2])
                        conf3 = conf.unsqueeze(2).broadcast_to([P, k, 3])
                        conf20 = conf.unsqueeze(2).broadcast_to([P, k, 20])

                        def slot(term):
                            i = ci * NTERMS + term
                            return acc[:, i:i + 1]

                        # -------- tiles --------
                        V = pv.tile([P, k, 3, 5], F32, name="V", tag="V")[:]
                        Vc = pvc.tile([P, k, 20], F32, name="Vc", tag="Vc")[:]
                        dwt = pw_pool.tile([P, k, 3, 4], F32, name="dwt",
                                           tag="dwt")[:]
                        sp = psqrt.tile([P, k, 3, 2], F32, name="sp", tag="sp")[:]
                        st = psqrt.tile([P, k, 3, 2], F32, name="st", tag="st")[:]
                        dcx = p6.tile([P, k, 3, 2], F32, name="dcx", tag="dcx")[:]
                        spt = p6.tile([P, k, 3, 2], F32, name="spt", tag="spt")[:]
                        m = p6.tile([P, k, 3, 2], F32, name="m", tag="m")[:]
                        inter = p3.tile([P, k, 3], F32, name="inter", tag="inter")[:]
                        a14 = p3.tile([P, k, 3], F32, name="a14", tag="a14")[:]
                        a24 = p1.tile([P, k], F32, name="a24", tag="a24")[:]
                        rcp = p3.tile([P, k, 3], F32, name="rcp", tag="rcp")[:]
                        miou = p1.tile([P, k], F32, name="miou", tag="miou")[:]
                        e = p3.tile([P, k, 3], F32, name="e", tag="e")[:]
                        rm5 = p3.tile([P, k, 3], F32, name="rm5", tag="rm5")[:]
                        dc = p3.tile([P, k, 3], F32, name="dc", tag="dc")[:]
                        sqp = p3.tile([P, k, 3], F32, name="sqp", tag="sqp")[:]
                        nm = p1.tile([P, k], F32, name="nm", tag="nm")[:]

                        a24b = a24.unsqueeze(2).broadcast_to([P, k, 3])
                        mioub = miou.unsqueeze(2).broadcast_to([P, k, 3])
                        nm3b = nm.unsqueeze(2).broadcast_to([P, k, 3])
                        rm5b4 = rm5.unsqueeze(3).broadcast_to([P, k, 3, 4])

                        # -------- ACT: early unary work --------
                        nc.scalar.activation(sp, pwh, AF.Sqrt)
                        nc.scalar.activation(st, twh, AF.Sqrt)
                        nc.scalar.activation(sqp, pc_, AF.Square)

                        # -------- Pool: independent of the DVE chain --------
                        nc.gpsimd.tensor_sub(dwt[:, :, :, 0:2], pxy,
                                             tb[:, :, :, 0:2])
                        nc.gpsimd.tensor_mul(a14, ob[:, :, :, 2], ob[:, :, :, 3])
                        nc.gpsimd.tensor_mul(a24, tw0, th0)
                        nc.gpsimd.tensor_add(a14, a14, a24b)     # s, in place
                        nc.gpsimd.tensor_sub(dwt[:, :, :, 2:4], sp, st)
                        nc.gpsimd.tensor_sub(Vc, ocls, tcls)
                        # Vc *= conf (obj mask), in place
                        nc.gpsimd.tensor_mul(Vc, Vc, conf20)

                        # -------- DVE: IoU / responsibility chain --------
                        # nm = 0.5 * (conf != 1)  (noobj weight folded in)
                        if use_ts2:
                            nc.vector.tensor_scalar(nm, conf, 1.0, 0.5,
                                                    op0=OP.not_equal, op1=OP.mult)
                        else:
                            nc.vector.tensor_scalar(nm, conf, 1.0, None,
                                                    op0=OP.not_equal)
                            nc.vector.tensor_scalar_mul(nm, nm, 0.5)
                        nc.vector.tensor_sub(dcx, pxy, txy0b)
                        nc.scalar.activation(dcx, dcx, AF.Abs, scale=2.0 / S)
                        nc.vector.tensor_add(spt, pwh, twh0b)
                        nc.vector.tensor_sub(spt, spt, dcx)      # u, in place
                        nc.vector.tensor_tensor(m, pwh, twh0b, op=OP.min)
                        nc.vector.scalar_tensor_tensor(
                            m, m, 2.0, spt, op0=OP.mult, op1=OP.min)
                        nc.scalar.activation(m, m, AF.Relu)
                        nc.vector.tensor_mul(inter, m[:, :, :, 0], m[:, :, :, 1])
                        nc.vector.scalar_tensor_tensor(          # den4, in place
                            a14, a14, 4.0, inter, op0=OP.mult, op1=OP.subtract)
                        nc.vector.reciprocal(rcp, a14)
                        nc.vector.tensor_mul(inter, inter, rcp)  # iou, in place
                        if use_reduce:
                            nc.vector.tensor_reduce(miou, inter, axis=AX.X,
                                                    op=OP.max)
                            nc.vector.tensor_tensor(e, inter, mioub,
                                                    op=OP.is_equal)
                            nc.vector.tensor_mul(e, e, conf3)    # resp, in place
                        else:
                            # baseline-style pairwise argmax one-hot
                            i0, i1, i2 = (inter[:, :, 0], inter[:, :, 1],
                                          inter[:, :, 2])
                            c01 = p1.tile([P, k], F32, name="c01", tag="c01")[:]
                            nc.vector.tensor_tensor(c01, i0, i1, op=OP.is_ge)
                            c02 = p1.tile([P, k], F32, name="c02", tag="c02")[:]
                            nc.vector.tensor_tensor(c02, i0, i2, op=OP.is_ge)
                            c12 = p1.tile([P, k], F32, name="c12", tag="c12")[:]
                            nc.vector.tensor_tensor(c12, i1, i2, op=OP.is_ge)
                            r0, r1, r2 = e[:, :, 0], e[:, :, 1], e[:, :, 2]
                            nc.vector.tensor_mul(r0, c01, c02)
                            nc.vector.scalar_tensor_tensor(
                                r1, r0, 1.0, c12, op0=OP.not_equal, op1=OP.mult)
                            t01 = p1.tile([P, k], F32, name="t01", tag="t01")[:]
                            nc.vector.tensor_add(t01, r0, r1)
                            nc.vector.tensor_scalar(
                                r2, t01, 1.0, None, op0=OP.not_equal)
                            nc.vector.tensor_mul(e, e, conf3)
                        nc.vector.tensor_scalar_mul(rm5, e, SQRT5)
                        nc.vector.tensor_mul(V[:, :, :, 0:4], dwt, rm5b4)
                        if use_reduce:
                            nc.vector.tensor_sub(dc, pc_, mioub)
                        else:
                            nc.vector.tensor_sub(dc, pc_, inter)
                        nc.vector.tensor_mul(V[:, :, :, 4], dc, e)
                        # noobj accumulate: sum(0.5*noobj*pc^2)
                        if use_ttr:
                            nc.vector.tensor_tensor_reduce(
                                sqp, sqp, nm3b, 1.0, 0.0,
                                op0=OP.mult, op1=OP.add, accum_out=slot(2))
                        else:
                            nc.vector.tensor_mul(sqp, sqp, nm3b)
                            nc.vector.tensor_reduce(slot(2), sqp, axis=AX.XY,
                                                    op=OP.add)

                        # -------- ACT: close the chunk --------
                        nc.scalar.activation(V, V, AF.Square, accum_out=slot(0))
                        nc.scalar.activation(Vc, Vc, AF.Square, accum_out=slot(1))

            nc.sync.dma_start(acc_h.ap()[:], acc[:])

    nc.compile()
    return nc


_CACHE = {}


def _get_nc(bc, ks=None, repeats=1, io_bufs=2, loop_repeats=0):
    key = (bc, tuple(ks) if ks else None, repeats, io_bufs, loop_repeats)
    if key not in _CACHE:
        _CACHE[key] = build_nc(bc, ks, repeats, io_bufs, loop_repeats)
    return _CACHE[key]


def combine_acc(acc_list, nchunks):
    tot = 0.0
    for a in acc_list:
        tot += a.astype(np.float64).sum()
    return np.float32(tot / B)


BEST_KS = [98, 147, 147]
BEST_IO_BUFS = 2


def kernel(output: np.ndarray, target: np.ndarray) -> np.ndarray:
    assert output.shape == (B, S, S, D) and target.shape == (B, S, S, D)
    bc = B // NCORES
    nchunks = len(BEST_KS)
    nc = _get_nc(bc, BEST_KS, io_bufs=BEST_IO_BUFS)
    in_maps = [
        {
            "output": np.ascontiguousarray(output[i * bc:(i + 1) * bc]),
            "target": np.ascontiguousarray(target[i * bc:(i + 1) * bc]),
        }
        for i in range(NCORES)
    ]
    res = run_bass_kernel_spmd(nc, in_maps, list(range(NCORES)))
    return combine_acc([r["acc"] for r in res.results], nchunks)


# revision 19
# speedup vs baseline: 1.1654x; 1.0775x over previous
"""YOLO-style DetectionLoss on 8 Trainium2 NeuronCores (Bass/Tile).

Pure data parallelism: batch 8192 -> 1024 per core; 1024*7*7 = 50176
cells laid out as 128 SBUF partitions x 392 cells (each partition owns 8
consecutive images). Per chunk of k cells the kernel builds masked
residual tiles whose squares sum to the loss:

  Vbox[...,b,0:4] = sqrt(5)*resp_b*(dxy | dwh)   (xy + wh terms)
  Vbox[...,b,4]   = resp_b*(pc_b - max_iou)      (contain term)
  Vcls[...,c]     = obj*(pcls_c - tcls_c)        (class term)
  noobj term      = 0.5*noobj*pc_b^2 via Square(pc)+masked accumulate

All loss weights are folded into the masks, so each chunk closes with
two ACT Square+accumulate ops (Vbox, Vcls) plus one Pool masked
accumulate -> 3 accumulator slots per chunk, summed on the host and
divided by the global batch.

The responsible-box one-hot uses reduce_max + is_equal (exact fp match)
instead of pairwise compares; ties can only occur when every IoU in a
cell is exactly 0 (measure-zero effect on the loss).
"""

import os

os.environ.setdefault("JAX_COMPILATION_CACHE_DIR", "/tmp/jaxcache")
os.environ.setdefault("JAX_PERSISTENT_CACHE_MIN_COMPILE_TIME_SECS", "1")
os.environ.setdefault("JAX_PERSISTENT_CACHE_MIN_ENTRY_SIZE_BYTES", "0")

import numpy as np

import concourse.bacc as bacc
import concourse.mybir as mybir
import concourse.tile as tile
from concourse.bass_utils import run_bass_kernel_spmd

F32 = mybir.dt.float32
AF = mybir.ActivationFunctionType
OP = mybir.AluOpType
AX = mybir.AxisListType

NB, C, S = 3, 20, 7
D = 5 * NB + C                 # 35
B = 8192
NCORES = 8
P = 128

SQRT5 = 5.0 ** 0.5
NTERMS = 3                     # box(xy+wh+contain), class, noobj


def default_chunks(kpp):
    if kpp == 392:
        return [98, 147, 147]
    if kpp % 98 == 0:
        return [98] * (kpp // 98)
    if kpp % 49 == 0:
        return [49] * (kpp // 49)
    return [kpp]


def build_nc(bc: int, ks=None, repeats: int = 1, io_bufs: int = 2,
             loop_repeats: int = 0, use_reduce: bool = True,
             use_ttr: bool = False, use_ts2: bool = True,
             cw_pool: bool = False, sq_scale: bool = True):
    """Trace the per-core Bass program for a per-core batch of `bc`."""
    cells = bc * S * S
    assert cells % P == 0
    kpp = cells // P
    if ks is None:
        ks = default_chunks(kpp)
    assert sum(ks) == kpp
    nchunks = len(ks)

    nc = bacc.Bacc("TRN2", debug=False, num_devices=NCORES)
    out_h = nc.dram_tensor("output", [bc, S, S, D], F32, kind="ExternalInput")
    tgt_h = nc.dram_tensor("target", [bc, S, S, D], F32, kind="ExternalInput")
    acc_h = nc.dram_tensor("acc", [P, NTERMS * nchunks], F32,
                           kind="ExternalOutput")

    out_v = out_h.ap().rearrange("(p a) h w d -> p (a h w d)", p=P)
    tgt_v = tgt_h.ap().rearrange("(p a) h w d -> p (a h w d)", p=P)

    with tile.TileContext(nc) as tc:
        with (
            tc.tile_pool(name="io", bufs=io_bufs) as io_pool,
            tc.tile_pool(name="pv", bufs=2) as pv,       # Vbox
            tc.tile_pool(name="pvc", bufs=2) as pvc,     # Vcls
            tc.tile_pool(name="p6", bufs=2) as p6,       # [k,3,2] temps
            tc.tile_pool(name="pw", bufs=2) as pw_pool,  # dwt [k,3,4]
            tc.tile_pool(name="psqrt", bufs=2) as psqrt, # sp/st
            tc.tile_pool(name="p3", bufs=2) as p3,       # [k,3] temps
            tc.tile_pool(name="p1", bufs=2) as p1,       # [k] temps
            tc.tile_pool(name="accp", bufs=1) as accp,
        ):
            acc = accp.tile([P, NTERMS * nchunks], F32)

            import contextlib
            loop_cm = (tc.For_i(0, loop_repeats, 1) if loop_repeats
                       else contextlib.nullcontext())
            with loop_cm:
                for rep in range(repeats):
                    off = 0
                    for ci, k in enumerate(ks):
                        ot = io_pool.tile([P, k * D], F32, name="ot", tag="ot")
                        tt = io_pool.tile([P, k * D], F32, name="tt", tag="tt")
                        nc.sync.dma_start(ot[:], out_v[:, off:off + k * D])
                        nc.sync.dma_start(tt[:], tgt_v[:, off:off + k * D])
                        off += k * D

                        o3 = ot[:].rearrange("p (k d) -> p k d", d=D)
                        t3 = tt[:].rearrange("p (k d) -> p k d", d=D)
                        ob = o3[:, :, 0:15].rearrange("p k (b f) -> p k b f", f=5)
                        tb = t3[:, :, 0:15].rearrange("p k (b f) -> p k b f", f=5)

                        pxy = ob[:, :, :, 0:2]
                        pwh = ob[:, :, :, 2:4]
                        pc_ = ob[:, :, :, 4]
                        twh = tb[:, :, :, 2:4]
                        t0 = tb[:, :, 0, :]
                        tw0 = t3[:, :, 2]
                        th0 = t3[:, :, 3]
                        conf = t3[:, :, 4]
                        ocls = o3[:, :, 15:35]
                        tcls = t3[:, :, 15:35]

                        txy0b = t0[:, :, 0:2].unsqueeze(2).broadcast_to(
                            [P, k, 3, 2])
                        twh0b = t0[:, :, 2:4].unsqueeze(2).broadcast_to(
                            [P, k, 3, 2])
                        conf3 = conf.unsqueeze(2).broadcast_to([P, k, 3])
                        conf20 = conf.unsqueeze(2).broadcast_to([P, k, 20])

                        def slot(term):
                            i = ci * NTERMS + term
                            return acc[:, i:i + 1]

                        # -------- tiles --------
                        V = pv.tile([P, k, 3, 5], F32, name="V", tag="V")[:]
                        Vc = pvc.tile([P, k, 20], F32, name="Vc", tag="Vc")[:]
                        dwt = pw_pool.tile([P, k, 3, 4], F32, name="dwt",
                                           tag="dwt")[:]
                        sp = psqrt.tile([P, k, 3, 2], F32, name="sp", tag="sp")[:]
                        st = psqrt.tile([P, k, 3, 2], F32, name="st", tag="st")[:]
                        dcx = p6.tile([P, k, 3, 2], F32, name="dcx", tag="dcx")[:]
                        spt = p6.tile([P, k, 3, 2], F32, name="spt", tag="spt")[:]
                        m = p6.tile([P, k, 3, 2], F32, name="m", tag="m")[:]
                        inter = p3.tile([P, k, 3], F32, name="inter", tag="inter")[:]
                        a1 = p3.tile([P, k, 3], F32, name="a1", tag="a1")[:]
                        s4 = p3.tile([P, k, 3], F32, name="s4", tag="s4")[:]
                        a24 = p1.tile([P, k], F32, name="a24", tag="a24")[:]
                        rcp = p3.tile([P, k, 3], F32, name="rcp", tag="rcp")[:]
                        miou = p1.tile([P, k], F32, name="miou", tag="miou")[:]
                        e = p3.tile([P, k, 3], F32, name="e", tag="e")[:]
                        rm5 = p3.tile([P, k, 3], F32, name="rm5", tag="rm5")[:]
                        dc = p3.tile([P, k, 3], F32, name="dc", tag="dc")[:]
                        pcm = p3.tile([P, k, 3], F32, name="pcm", tag="pcm")[:]
                        nm = p1.tile([P, k], F32, name="nm", tag="nm")[:]

                        a24b = a24.unsqueeze(2).broadcast_to([P, k, 3])
                        mioub = miou.unsqueeze(2).broadcast_to([P, k, 3])
                        nm3b = nm.unsqueeze(2).broadcast_to([P, k, 3])
                        rm5b4 = rm5.unsqueeze(3).broadcast_to([P, k, 3, 4])

                        # -------- ACT: early unary work --------
                        nc.scalar.activation(sp, pwh, AF.Sqrt)
                        nc.scalar.activation(st, twh, AF.Sqrt)

                        # -------- DVE: nm first (Pool pcm needs it) --------
                        # nm = 0.5 * (conf != 1)  (noobj weight folded in)
                        nc.vector.tensor_scalar(nm, conf, 1.0, 0.5,
                                                op0=OP.not_equal, op1=OP.mult)

                        # -------- Pool: no in-place writes (drain-free) ----
                        nc.gpsimd.tensor_sub(dwt[:, :, :, 0:2], pxy,
                                             tb[:, :, :, 0:2])
                        nc.gpsimd.tensor_mul(a1, ob[:, :, :, 2], ob[:, :, :, 3])
                        nc.gpsimd.tensor_mul(a24, tw0, th0)
                        nc.gpsimd.tensor_add(s4, a1, a24b)
                        nc.gpsimd.tensor_sub(dwt[:, :, :, 2:4], sp, st)
                        nc.gpsimd.tensor_sub(Vc, ocls, tcls)
                        # pcm = nm * pc  ((sqrt2*pcm)^2 = 0.5*noobj*pc^2)
                        nc.gpsimd.tensor_mul(pcm, pc_, nm3b)

                        # -------- DVE: IoU / responsibility chain --------
                        nc.vector.tensor_sub(dcx, pxy, txy0b)
                        nc.scalar.activation(dcx, dcx, AF.Abs, scale=2.0 / S)
                        nc.vector.tensor_add(spt, pwh, twh0b)
                        nc.vector.tensor_sub(spt, spt, dcx)      # u, in place
                        nc.vector.tensor_tensor(m, pwh, twh0b, op=OP.min)
                        nc.vector.scalar_tensor_tensor(
                            m, m, 2.0, spt, op0=OP.mult, op1=OP.min)
                        nc.scalar.activation(m, m, AF.Relu)
                        nc.vector.tensor_mul(inter, m[:, :, :, 0], m[:, :, :, 1])
                        nc.vector.scalar_tensor_tensor(          # den4, in place
                            s4, s4, 4.0, inter, op0=OP.mult, op1=OP.subtract)
                        nc.vector.reciprocal(rcp, s4)
                        nc.vector.tensor_mul(inter, inter, rcp)  # iou, in place
                        if use_reduce:
                            nc.vector.tensor_reduce(miou, inter, axis=AX.X,
                                                    op=OP.max)
                            nc.vector.tensor_tensor(e, inter, mioub,
                                                    op=OP.is_equal)
                            nc.vector.tensor_mul(e, e, conf3)    # resp, in place
                        else:
                            i0, i1, i2 = (inter[:, :, 0], inter[:, :, 1],
                                          inter[:, :, 2])
                            c01 = p1.tile([P, k], F32, name="c01", tag="c01")[:]
                            nc.vector.tensor_tensor(c01, i0, i1, op=OP.is_ge)
                            c02 = p1.tile([P, k], F32, name="c02", tag="c02")[:]
                            nc.vector.tensor_tensor(c02, i0, i2, op=OP.is_ge)
                            c12 = p1.tile([P, k], F32, name="c12", tag="c12")[:]
                            nc.vector.tensor_tensor(c12, i1, i2, op=OP.is_ge)
                            r0, r1, r2 = e[:, :, 0], e[:, :, 1], e[:, :, 2]
                            nc.vector.tensor_mul(r0, c01, c02)
                            nc.vector.scalar_tensor_tensor(
                                r1, r0, 1.0, c12, op0=OP.not_equal, op1=OP.mult)
                            t01 = p1.tile([P, k], F32, name="t01", tag="t01")[:]
                            nc.vector.tensor_add(t01, r0, r1)
                            nc.vector.tensor_scalar(
                                r2, t01, 1.0, None, op0=OP.not_equal)
                            nc.vector.tensor_mul(e, e, conf3)
                        nc.vector.tensor_scalar_mul(rm5, e, SQRT5)
                        if use_reduce:
                            nc.vector.tensor_sub(dc, pc_, mioub)
                        else:
                            nc.vector.tensor_sub(dc, pc_, inter)
                        nc.vector.tensor_mul(V[:, :, :, 4], dc, e)
                        # class mask (in place on DVE; Pool wrote the sub)
                        nc.vector.tensor_mul(Vc, Vc, conf20)

                        # -------- masked box residuals --------
                        if cw_pool:
                            nc.gpsimd.tensor_mul(V[:, :, :, 0:4], dwt, rm5b4)
                        else:
                            nc.vector.tensor_mul(V[:, :, :, 0:4], dwt, rm5b4)

                        # -------- ACT: close the chunk --------
                        if sq_scale:
                            nc.scalar.activation(pcm, pcm, AF.Square,
                                                 scale=2.0 ** 0.5,
                                                 accum_out=slot(2))
                        else:
                            nc.vector.tensor_scalar_mul(pcm, pcm, 2.0 ** 0.5)
                            nc.scalar.activation(pcm, pcm, AF.Square,
                                                 accum_out=slot(2))
                        nc.scalar.activation(V, V, AF.Square, accum_out=slot(0))
                        nc.scalar.activation(Vc, Vc, AF.Square, accum_out=slot(1))

            nc.sync.dma_start(acc_h.ap()[:], acc[:])

    nc.compile()
    return nc


_CACHE = {}


def _get_nc(bc, ks=None, repeats=1, io_bufs=2, loop_repeats=0):
    key = (bc, tuple(ks) if ks else None, repeats, io_bufs, loop_repeats)
    if key not in _CACHE:
        _CACHE[key] = build_nc(bc, ks, repeats, io_bufs, loop_repeats)
    return _CACHE[key]


def combine_acc(acc_list, nchunks):
    tot = 0.0
    for a in acc_list:
        tot += a.astype(np.float64).sum()
    return np.float32(tot / B)


BEST_KS = [98, 147, 147]
BEST_IO_BUFS = 2


def kernel(output: np.ndarray, target: np.ndarray) -> np.ndarray:
    assert output.shape == (B, S, S, D) and target.shape == (B, S, S, D)
    bc = B // NCORES
    nchunks = len(BEST_KS)
    nc = _get_nc(bc, BEST_KS, io_bufs=BEST_IO_BUFS)
    in_maps = [
        {
            "output": np.ascontiguousarray(output[i * bc:(i + 1) * bc]),
            "target": np.ascontiguousarray(target[i * bc:(i + 1) * bc]),
        }
        for i in range(NCORES)
    ]
    res = run_bass_kernel_spmd(nc, in_maps, list(range(NCORES)))
    return combine_acc([r["acc"] for r in res.results], nchunks)
